# revision 8
# baseline (speedup 1.0000x reference)
"""DTW (symmetric2, L1 cost) batch kernel for Trainium2, 8 NeuronCores.

Problem: 64 pairs of length-1024 fp32 sequences; per pair the full
1024x1024 DTW dynamic program; output = mean over pairs of
D[n-1, m-1] / (n + m).

Raw-bass hand-scheduled implementation (no Tile framework):
  - Row-scan DP per core (8 samples, partition p = 8*chunk + sample):
        q[j] = Dprev[j-1] + d[j];  p[j] = min(q[j], Dprev[j])
        D[j] = min(p[j], D[j-1]) + d[j]   (DVE tensor_tensor_scan)
  - 16 column chunks of 64 in a software wavefront, R=4 rows per
    macro-step, SKEW=3 macro-steps of lag between adjacent chunks.
  - Same-engine RAW hazards are left to the DVE's in-order pipeline +
    drain (no per-op semaphores), so the q/p/scan chain streams at
    ~0.55 us per DP row-step; cross-engine sync happens once per
    macro-step.
  - Chunk boundary columns move via one SBUF->SBUF partition-shifted
    DMA per macro-step on the Sync queue. The DMA's semaphore posts
    16 progress bumps spread across the transfer (not a single
    completion bump), so the consumer waits one extra whole DMA
    (SM >= 16*(tau-SKEW+2)) for write-visibility settle; determinism
    verified against this (bit-exact across repeated runs).
  - Local-cost rows d = |y - x_i| come from the Scalar engine
    (activation, scale=-1, bias=x_i), one macro-step batch ahead.
"""

import sys

sys.path.insert(0, "/opt/trn_rl_repo")

import numpy as np

import concourse.bass as bass
import concourse.mybir as mybir
from concourse.bass_utils import run_bass_kernel_spmd

AF = mybir.ActivationFunctionType
ALU = mybir.AluOpType
FP32 = mybir.dt.float32

NCORES = 8
B = 8             # samples per core
N = 1024          # sequence length
C = 16            # column chunks
W = N // C        # 64
R = 4             # rows per macro-step
SKEW = 3          # macro-steps of lag between adjacent chunks
T = N // R + SKEW * (C - 1)   # 301
S_TOTAL = T * R               # 1204
NSLOT = 16
ND = 16           # d-row ring: 4 macro-step batches in flight
NLF = 4           # lf ring
BIG = 1.0e30

_CACHE = {}


def _build():
    nc = bass.Bass("TRN2", target_bir_lowering=False)

    x8 = nc.dram_tensor("x8", [B, N], FP32, kind="ExternalInput")
    y8 = nc.dram_tensor("y8", [B, N], FP32, kind="ExternalInput")
    out = nc.dram_tensor("dists", [B, 1], FP32, kind="ExternalOutput")

    with (
        nc.Block() as block,
        nc.semaphore("dma_in") as dma_in,
        nc.semaphore("sa") as SA,
        nc.semaphore("sd") as SD,
        nc.semaphore("sm") as SM,
        nc.semaphore("v_memset") as v_memset,
        nc.semaphore("dma_out") as dma_out,
        # hot steady-state tensors first: their SBUF offsets match the
        # layout that measured 515 ns/row-step
        nc.sbuf_tensor("Y", [128, W], FP32) as Y,
        nc.sbuf_tensor("BB", [128, NSLOT, W + 1], FP32) as BB,
        nc.sbuf_tensor("LF", [128, NLF, R + 1], FP32) as LF,
        nc.sbuf_tensor("DTS", [128, ND, W + 1], FP32) as DTS,
        nc.sbuf_tensor("PTS", [128, 3, W + 1], FP32) as PTS,
        nc.sbuf_tensor("QQ", [128, 2, W], FP32) as QQ,
        nc.sbuf_tensor("ZC", [128, 1], FP32) as ZC,
        nc.sbuf_tensor("XS", [128, S_TOTAL], FP32) as XS,
    ):
        N_IN_DMAS = C + 1  # 16 XS slices + 1 combined Y

        @block.sync
        def _(sync):
            sync.wait_ge(v_memset, 1)
            # Y[8c+b, j] = y[b, 64c+j]: one DMA, src iterates (c, b, j)
            sync.dma_start(
                Y[:, :], bass.AP(y8, 0, [[W, C], [N, B], [1, W]])
            ).then_inc(dma_in, 16)
            for c in range(C):
                o = SKEW * R * c
                sync.dma_start(XS[8 * c : 8 * c + 8, o : o + N], x8[:, :]).then_inc(
                    dma_in, 16
                )
            with nc.allow_non_contiguous_dma(reason="R-elem boundary gather"):
                for tau in range(T - SKEW):
                    sync.wait_ge(SD, tau + 1)
                    k0 = (R * tau) % NSLOT
                    sync.dma_start(
                        LF[8:128, (tau + SKEW) % NLF, 1 : R + 1],
                        BB[0:120, k0 : k0 + R, W : W + 1],
                    ).then_inc(SM, 16)
                sync.wait_ge(SD, T + 1)
                for _ in range(10):
                    sync.wait_ge(SD, T + 1)  # settle spin ~0.5us
                sync.dma_start(
                    out[:, :], BB[120:128, (S_TOTAL - 1) % NSLOT, W : W + 1]
                ).then_inc(dma_out, 16)
            sync.wait_ge(dma_out, 16)

        @block.scalar
        def _(scalar):
            scalar.wait_ge(v_memset, 1)

            def dma_gate(tau):
                need = min(2 + (R * tau + R - 1) // (SKEW * R), N_IN_DMAS)
                scalar.wait_ge(dma_in, 16 * need)

            def act_batch(tau):
                for r in range(R):
                    s = R * tau + r
                    ins = nc.scalar.activation(
                        DTS[:, s % ND, 1 : W + 1],
                        Y[:, :],
                        AF.Abs,
                        bias=XS[:, s : s + 1],
                        scale=-1.0,
                    )
                    if r == R - 1:
                        ins.then_inc(SA, 1)

            for tau in range(4):
                dma_gate(tau)
                act_batch(tau)
            for tau in range(4, T):
                dma_gate(tau)
                scalar.wait_ge(SD, tau - 3)
                act_batch(tau)

        @block.vector
        def _(vector):
            # acts need only XS pads + Y + per-chunk x slices; fire the
            # input-DMA gate right after the XS memset. The remaining
            # memsets complete before the DVE main loop (same queue) and
            # before any cross-engine reader (boundary DMA waits SD>=1).
            nc.vector.memset(XS[:, :], BIG).then_inc(v_memset, 1)
            nc.vector.memset(BB[:, :, :], BIG)
            nc.vector.memset(LF[:, :, :], BIG)
            nc.vector.memset(ZC[:, :], BIG)
            nc.vector.memset(ZC[0:8, :], 0.0)
            nc.vector.memset(DTS[:, :, 0:1], 0.0)
            nc.vector.memset(PTS[:, :, 0:1], BIG)

            for tau in range(T):
                # even macro-steps' waits cover the following odd one too;
                # odd-macro consumption gets its write-settle from a full
                # extra macro-step of wall time
                if tau % 2 == 0:
                    vector.wait_ge(SA, min(tau + 2, T))
                lf = LF[:, tau % NLF, :]
                for r in range(R):
                    s = R * tau + r
                    b_prev = BB[:, (s - 1) % NSLOT, :]
                    b_cur = BB[:, s % NSLOT, :]
                    d = DTS[:, s % ND, :]
                    q = QQ[:, s % 2, :]
                    p = PTS[:, s % 3, :]
                    nc.vector.tensor_tensor(
                        q[:, :], b_prev[:, 0:W], d[:, 1 : W + 1], op=ALU.add
                    )
                    nc.vector.tensor_tensor(
                        p[:, 1 : W + 1], q[:, :], b_prev[:, 1 : W + 1], op=ALU.min
                    )
                    if r == 0 and tau >= SKEW and (tau % 2 == 0 or tau == SKEW):
                        vector.wait_ge(SM, 16 * min(tau - SKEW + 2, T - SKEW))
                    if s == 0:
                        nc.vector.tensor_tensor_scan(
                            b_cur[:, 1 : W + 1],
                            p[:, 1 : W + 1],
                            d[:, 1 : W + 1],
                            ZC[:, 0:1],
                            op0=ALU.min,
                            op1=ALU.add,
                        )
                        ins = nc.vector.memset(b_cur[:, 0:1], BIG)
                    else:
                        ins = nc.vector.tensor_tensor_scan(
                            b_cur[:, 0 : W + 1],
                            p[:, 0 : W + 1],
                            d[:, 0 : W + 1],
                            lf[:, r + 1 : r + 2],
                            op0=ALU.min,
                            op1=ALU.add,
                        )
                    if r == R - 1:
                        ins.then_inc(SD, 1)

            # drain padding after the final scan: give its SBUF writes
            # time to land before the output DMA reads them
            nc.vector.memset(QQ[:, 0, :], 0.0)
            nc.vector.memset(QQ[:, 1, :], 0.0)
            nc.vector.memset(QQ[:, 0, :], 0.0).then_inc(SD, 1)

    return nc


LAST = {}


def kernel(x: np.ndarray, x_target: np.ndarray) -> np.ndarray:
    import os

    x = np.ascontiguousarray(np.asarray(x, np.float32))
    y = np.ascontiguousarray(np.asarray(x_target, np.float32))
    if "nc" not in _CACHE:
        _CACHE["nc"] = _build()
    nc = _CACHE["nc"]
    in_maps = [
        {"x8": x[8 * k : 8 * k + 8], "y8": y[8 * k : 8 * k + 8]}
        for k in range(NCORES)
    ]
    trace = bool(os.environ.get("DTW_TRACE"))
    r = run_bass_kernel_spmd(nc, in_maps, list(range(NCORES)), trace=trace)
    LAST["exec_time_ns"] = r.exec_time_ns
    LAST["profile_json"] = r.profile_json
    LAST["trace_path"] = (
        r.instructions_and_trace[1] if r.instructions_and_trace else None
    )
    res = r.results
    dists = np.concatenate([rr["dists"][:, 0] for rr in res]).astype(np.float32)
    dists = dists / np.float32(2.0 * N)
    return np.float32(np.mean(dists))


# revision 9
# speedup vs baseline: 1.0052x; 1.0052x over previous
"""DTW (symmetric2, L1 cost) batch kernel for Trainium2, 8 NeuronCores.

Problem: 64 pairs of length-1024 fp32 sequences; per pair the full
1024x1024 DTW dynamic program; output = mean over pairs of
D[n-1, m-1] / (n + m).

Raw-bass hand-scheduled implementation (no Tile framework):
  - Row-scan DP per core (8 samples, partition p = 8*chunk + sample):
        q[j] = Dprev[j-1] + d[j];  p[j] = min(q[j], Dprev[j])
        D[j] = min(p[j], D[j-1]) + d[j]   (DVE tensor_tensor_scan)
  - 16 column chunks of 64 in a software wavefront, R=4 rows per
    macro-step, SKEW=3 macro-steps of lag between adjacent chunks.
  - Same-engine RAW hazards are left to the DVE's in-order pipeline +
    drain (no per-op semaphores), so the q/p/scan chain streams at
    ~0.55 us per DP row-step; cross-engine sync happens once per
    macro-step.
  - Chunk boundary columns move via one SBUF->SBUF partition-shifted
    DMA per macro-step on the Sync queue. The DMA's semaphore posts
    16 progress bumps spread across the transfer (not a single
    completion bump), so the consumer waits one extra whole DMA
    (SM >= 16*(tau-SKEW+2)) for write-visibility settle; determinism
    verified against this (bit-exact across repeated runs).
  - Local-cost rows d = |y - x_i| come from the Scalar engine
    (activation, scale=-1, bias=x_i), up to three macro-step batches
    ahead (ND=16 ring). Cross-engine waits are emitted only at even
    macro-steps (covering the following odd one); the odd macro's lf
    write-settle comes free from the extra macro-step of wall time.
"""

import sys

sys.path.insert(0, "/opt/trn_rl_repo")

import numpy as np

import concourse.bass as bass
import concourse.mybir as mybir
from concourse.bass_utils import run_bass_kernel_spmd

AF = mybir.ActivationFunctionType
ALU = mybir.AluOpType
FP32 = mybir.dt.float32

NCORES = 8
B = 8             # samples per core
N = 1024          # sequence length
C = 16            # column chunks
W = N // C        # 64
R = 4             # rows per macro-step
SKEW = 3          # macro-steps of lag between adjacent chunks
T = N // R + SKEW * (C - 1)   # 301
S_TOTAL = T * R               # 1204
NSLOT = 16
ND = 16           # d-row ring: 4 macro-step batches in flight
NLF = 4           # lf ring
BIG = 1.0e30

_CACHE = {}


def _build():
    nc = bass.Bass("TRN2", target_bir_lowering=False)

    x8 = nc.dram_tensor("x8", [B, N], FP32, kind="ExternalInput")
    y8 = nc.dram_tensor("y8", [B, N], FP32, kind="ExternalInput")
    out = nc.dram_tensor("dists", [B, 1], FP32, kind="ExternalOutput")

    with (
        nc.Block() as block,
        nc.semaphore("dma_in") as dma_in,
        nc.semaphore("sa") as SA,
        nc.semaphore("sd") as SD,
        nc.semaphore("sm") as SM,
        nc.semaphore("v_memset") as v_memset,
        nc.semaphore("dma_out") as dma_out,
        # hot steady-state tensors first: their SBUF offsets match the
        # layout that measured 515 ns/row-step
        nc.sbuf_tensor("Y", [128, W], FP32) as Y,
        nc.sbuf_tensor("BB", [128, NSLOT, W + 1], FP32) as BB,
        nc.sbuf_tensor("LF", [128, NLF, R + 1], FP32) as LF,
        nc.sbuf_tensor("DTS", [128, ND, W + 1], FP32) as DTS,
        nc.sbuf_tensor("PTS", [128, 3, W + 1], FP32) as PTS,
        nc.sbuf_tensor("QQ", [128, 2, W], FP32) as QQ,
        nc.sbuf_tensor("ZC", [128, 1], FP32) as ZC,
        nc.sbuf_tensor("XS", [128, S_TOTAL], FP32) as XS,
    ):
        N_IN_DMAS = C + 1  # 16 XS slices + 1 combined Y

        @block.sync
        def _(sync):
            sync.wait_ge(v_memset, 1)
            # Y[8c+b, j] = y[b, 64c+j]: one DMA, src iterates (c, b, j)
            sync.dma_start(
                Y[:, :], bass.AP(y8, 0, [[W, C], [N, B], [1, W]])
            ).then_inc(dma_in, 16)
            for c in range(C):
                o = SKEW * R * c
                sync.dma_start(XS[8 * c : 8 * c + 8, o : o + N], x8[:, :]).then_inc(
                    dma_in, 16
                )
            with nc.allow_non_contiguous_dma(reason="R-elem boundary gather"):
                for tau in range(T - SKEW):
                    sync.wait_ge(SD, tau + 1)
                    k0 = (R * tau) % NSLOT
                    sync.dma_start(
                        LF[8:128, (tau + SKEW) % NLF, 1 : R + 1],
                        BB[0:120, k0 : k0 + R, W : W + 1],
                    ).then_inc(SM, 16)
                sync.wait_ge(SD, T + 1)
                for _ in range(10):
                    sync.wait_ge(SD, T + 1)  # settle spin ~0.5us
                sync.dma_start(
                    out[:, :], BB[120:128, (S_TOTAL - 1) % NSLOT, W : W + 1]
                ).then_inc(dma_out, 16)
            sync.wait_ge(dma_out, 16)

        @block.scalar
        def _(scalar):
            scalar.wait_ge(v_memset, 1)

            def dma_gate(tau):
                need = min(2 + (R * tau + R - 1) // (SKEW * R), N_IN_DMAS)
                scalar.wait_ge(dma_in, 16 * need)

            def act_batch(tau):
                for r in range(R):
                    s = R * tau + r
                    ins = nc.scalar.activation(
                        DTS[:, s % ND, 1 : W + 1],
                        Y[:, :],
                        AF.Abs,
                        bias=XS[:, s : s + 1],
                        scale=-1.0,
                    )
                    if r == R - 1:
                        ins.then_inc(SA, 1)

            for tau in range(4):
                dma_gate(tau)
                act_batch(tau)
            for tau in range(4, T):
                dma_gate(tau)
                scalar.wait_ge(SD, tau - 3)
                act_batch(tau)

        @block.vector
        def _(vector):
            # acts need only XS pads + Y + per-chunk x slices; fire the
            # input-DMA gate right after the XS memset. The remaining
            # memsets complete before the DVE main loop (same queue) and
            # before any cross-engine reader (boundary DMA waits SD>=1).
            nc.vector.memset(XS[:, :], BIG).then_inc(v_memset, 1)
            nc.vector.memset(BB[:, :, :], BIG)
            nc.vector.memset(LF[:, :, :], BIG)
            nc.vector.memset(ZC[:, :], BIG)
            nc.vector.memset(ZC[0:8, :], 0.0)
            nc.vector.memset(DTS[:, :, 0:1], 0.0)
            nc.vector.memset(PTS[:, :, 0:1], BIG)

            for tau in range(T):
                # even macro-steps' waits cover the following odd one too;
                # odd-macro consumption gets its write-settle from a full
                # extra macro-step of wall time
                if tau % 2 == 0:
                    vector.wait_ge(SA, min(tau + 2, T))
                lf = LF[:, tau % NLF, :]
                for r in range(R):
                    s = R * tau + r
                    b_prev = BB[:, (s - 1) % NSLOT, :]
                    b_cur = BB[:, s % NSLOT, :]
                    d = DTS[:, s % ND, :]
                    q = QQ[:, s % 2, :]
                    p = PTS[:, s % 3, :]
                    nc.vector.tensor_tensor(
                        q[:, :], b_prev[:, 0:W], d[:, 1 : W + 1], op=ALU.add
                    )
                    nc.vector.tensor_tensor(
                        p[:, 1 : W + 1], q[:, :], b_prev[:, 1 : W + 1], op=ALU.min
                    )
                    if r == 0 and tau >= SKEW and (tau % 2 == 0 or tau == SKEW):
                        vector.wait_ge(SM, 16 * min(tau - SKEW + 2, T - SKEW))
                    if s == 0:
                        nc.vector.tensor_tensor_scan(
                            b_cur[:, 1 : W + 1],
                            p[:, 1 : W + 1],
                            d[:, 1 : W + 1],
                            ZC[:, 0:1],
                            op0=ALU.min,
                            op1=ALU.add,
                        )
                        ins = nc.vector.memset(b_cur[:, 0:1], BIG)
                    else:
                        ins = nc.vector.tensor_tensor_scan(
                            b_cur[:, 0 : W + 1],
                            p[:, 0 : W + 1],
                            d[:, 0 : W + 1],
                            lf[:, r + 1 : r + 2],
                            op0=ALU.min,
                            op1=ALU.add,
                        )
                    if r == R - 1:
                        ins.then_inc(SD, 1)

            # drain padding after the final scan: give its SBUF writes
            # time to land before the output DMA reads them
            nc.vector.memset(QQ[:, 0, :], 0.0)
            nc.vector.memset(QQ[:, 1, :], 0.0)
            nc.vector.memset(QQ[:, 0, :], 0.0).then_inc(SD, 1)

    return nc


LAST = {}


def kernel(x: np.ndarray, x_target: np.ndarray) -> np.ndarray:
    import os

    x = np.ascontiguousarray(np.asarray(x, np.float32))
    y = np.ascontiguousarray(np.asarray(x_target, np.float32))
    if "nc" not in _CACHE:
        _CACHE["nc"] = _build()
    nc = _CACHE["nc"]
    in_maps = [
        {"x8": x[8 * k : 8 * k + 8], "y8": y[8 * k : 8 * k + 8]}
        for k in range(NCORES)
    ]
    trace = bool(os.environ.get("DTW_TRACE"))
    r = run_bass_kernel_spmd(nc, in_maps, list(range(NCORES)), trace=trace)
    LAST["exec_time_ns"] = r.exec_time_ns
    LAST["profile_json"] = r.profile_json
    LAST["trace_path"] = (
        r.instructions_and_trace[1] if r.instructions_and_trace else None
    )
    res = r.results
    dists = np.concatenate([rr["dists"][:, 0] for rr in res]).astype(np.float32)
    dists = dists / np.float32(2.0 * N)
    return np.float32(np.mean(dists))
